# revision 1
# baseline (speedup 1.0000x reference)
"""MultiHeadAttention Trainium2 Bass kernel (8 cores).

Problem: B=2, S=2048, D=1024, H=16 heads, DK=64, fp32.
  q/k/v = x @ W* + b*; scores = q k^T / 8; attn = softmax; ctx = attn v;
  out = ctx @ Wo + bo.

Sharding (8 cores): batch (2-way) x head-group (4-way tensor parallel).
Core c handles b = c // 4 and heads [4g, 4g+4), g = c % 4 (d' slice of 256).
Each core gets x[b]^T and the W column/row slices for its head group, computes
a partial out [S, D] (contraction over its 256 d' rows of Wo), and the host
sums the 4 partials per batch and adds the host-folded bias correction
(bv @ Wo + bo).

On-device layout: "transposed activations". qT/kT [256, S] (d' on
partitions), v natural [S, 256+ones]. Attention per (head, qi-chunk):
  scoresT[kj, qi] = kT^T qT   (PE, fp32r)
  attnT = exp(scoresT / 8)    (ACT, psum->sbuf, fp32r out)
  ctxT[d'+sums, qi] += v_aug^T attnT  (PE; ones col in v gives row sums)
  ctxT /= sums  (partition_broadcast + reciprocal_approx_fast + DVE mul)
out-proj: out[s, :] = sum_mt ctxT[:, mt, s]^T wo[mt]  (PE), DVE drain, DMA.

All matmul operands are float32r (fp32 bits, PE rounds internally; 1 cyc/row
at N>=256 vs 4 cyc/row for exact fp32; measured matmul rel err ~1.5e-4).

The projection matmuls are interleaved into the attention loop (a prologue
computes just enough of kT/qT to start head 0; the rest drips in between
attention steps) so the ACT engine -- the bottleneck (16.8M exps/core) --
starts almost immediately and stays fed.
"""

import numpy as np

B = 2
S = 2048
D = 1024
H = 16
DK = 64
N_CORES = 8
HL = H // 4  # 4 heads per core
DL = HL * DK  # 256 local d'
QC = 1024  # qi chunk for scores/exp
KJT = S // 128  # 16 kj tiles
KT = D // 128  # 8 contraction tiles for projections

_CACHED_NC = None


def _build():
    import concourse.bacc as bacc
    import concourse.mybir as mybir
    import concourse.tile as tile

    f32 = mybir.dt.float32
    f32r = mybir.dt.float32r
    Exp = mybir.ActivationFunctionType.Exp

    nc = bacc.Bacc(None)

    xT = nc.declare_dram_parameter("xT", [D, S], f32r, isOutput=False)
    wq = nc.declare_dram_parameter("wq", [D, DL], f32r, isOutput=False)
    wk = nc.declare_dram_parameter("wk", [D, DL], f32r, isOutput=False)
    wv = nc.declare_dram_parameter("wv", [D, DL], f32r, isOutput=False)
    wo = nc.declare_dram_parameter("wo", [DL, D], f32r, isOutput=False)
    bq = nc.declare_dram_parameter("bq", [128, 2], f32, isOutput=False)
    bk = nc.declare_dram_parameter("bk", [128, 2], f32, isOutput=False)
    out = nc.declare_dram_parameter("out", [S, D], f32, isOutput=True)

    with tile.TileContext(nc) as tc:
        with (
            tc.tile_pool(name="persist", bufs=1) as persist,
            tc.tile_pool(name="ph1", bufs=1) as ph1,
            tc.tile_pool(name="attn", bufs=4) as atp,
            tc.tile_pool(name="norm", bufs=2) as npl,
            tc.tile_pool(name="ob", bufs=2) as obp,
            tc.tile_pool(name="scps", bufs=2, space="PSUM") as scp,
            tc.tile_pool(name="wsps", bufs=2, space="PSUM") as wsp,
            tc.tile_pool(name="cxps", bufs=2, space="PSUM") as cxp,
        ):
            qT_sb = persist.tile([128, 2, S], f32r, tag="qT")
            kT_sb = persist.tile([128, 2, S], f32r, tag="kT")
            v_sb = persist.tile([128, KJT, HL, DK + 1], f32r, tag="v")
            ctxT_sb = persist.tile([128, 2, S], f32r, tag="ctxT")
            wo_sb = persist.tile([128, 2, D], f32r, tag="wo")
            bq_sb = persist.tile([128, 2], f32, tag="bq")
            bk_sb = persist.tile([128, 2], f32, tag="bk")
            ones_f32 = persist.tile([128, KJT, HL, 1], f32, tag="ones")

            nc.sync.dma_start(out=bq_sb[:], in_=bq[:])
            nc.sync.dma_start(out=bk_sb[:], in_=bk[:])
            for mt in range(2):
                nc.sync.dma_start(
                    out=wo_sb[:, mt, :], in_=wo[mt * 128 : (mt + 1) * 128, :]
                )
            nc.vector.memset(ones_f32[:], 1.0)
            nc.vector.tensor_copy(v_sb[:, :, :, DK : DK + 1], ones_f32[:])

            xt, wq_t, wk_t, wv_t = [], [], [], []
            for kt in range(KT):
                t = ph1.tile([128, S], f32r, tag=f"xt{kt}")
                nc.sync.dma_start(out=t[:], in_=xT[kt * 128 : (kt + 1) * 128, :])
                xt.append(t)
                for nm, lst, prm in (
                    ("wq", wq_t, wq),
                    ("wk", wk_t, wk),
                    ("wv", wv_t, wv),
                ):
                    w = ph1.tile([128, DL], f32r, tag=f"{nm}{kt}")
                    nc.sync.dma_start(
                        out=w[:], in_=prm[kt * 128 : (kt + 1) * 128, :]
                    )
                    lst.append(w)

            def qk_chunk(which, mt, n):
                """Project one [128, 512] chunk of qT (which=0) / kT (which=1)."""
                wt, dst, bias = (
                    (wq_t, qT_sb, bq_sb) if which == 0 else (wk_t, kT_sb, bk_sb)
                )
                ns = slice(n * 512, (n + 1) * 512)
                ps = wsp.tile([128, 512], f32, tag="ws", name=f"pj{which}{mt}{n}")
                for kt in range(KT):
                    nc.tensor.matmul(
                        ps[:],
                        wt[kt][:, mt * 128 : (mt + 1) * 128],
                        xt[kt][:, ns],
                        start=(kt == 0),
                        stop=(kt == KT - 1),
                    )
                nc.vector.tensor_scalar_add(
                    out=dst[:, mt, ns], in0=ps[:], scalar1=bias[:, mt : mt + 1]
                )

            def v_chunk(jt):
                """Project v rows [jt*128, (jt+1)*128) for all 4 heads."""
                js = slice(jt * 128, (jt + 1) * 128)
                ps = wsp.tile([128, DL], f32, tag="ws", name=f"vp{jt}")
                for kt in range(KT):
                    nc.tensor.matmul(
                        ps[:],
                        xt[kt][:, js],
                        wv_t[kt][:],
                        start=(kt == 0),
                        stop=(kt == KT - 1),
                    )
                nc.vector.tensor_copy(
                    v_sb[:, jt, :, 0:DK],
                    ps[:].rearrange("p (h d) -> p h d", h=HL),
                )

            def out_proj_piece(st, nt, c, tail=False):
                s0 = c * 512 + st * 128
                op = wsp.tile([128, 512], f32, tag="ws", name=f"op{c}{st}{nt}")
                for mt2 in range(2):
                    nc.tensor.matmul(
                        op[:],
                        ctxT_sb[:, mt2, s0 : s0 + 128],
                        wo_sb[:, mt2, nt * 512 : (nt + 1) * 512],
                        start=(mt2 == 0),
                        stop=(mt2 == 1),
                    )
                ob = obp.tile([128, 512], f32, tag="ob")
                if tail and (st + nt) % 2 == 0:
                    nc.scalar.copy(ob[:], op[:])  # ACT is idle in the tail
                else:
                    nc.vector.tensor_copy(ob[:], op[:])
                nc.sync.dma_start(
                    out=out[s0 : s0 + 128, nt * 512 : (nt + 1) * 512],
                    in_=ob[:],
                )

            # Prologue: just enough for chunk-phase (c=0, mt=0) to start.
            qk_chunk(1, 0, 0)  # kT mt0 n0 (kj tiles 0-3)
            qk_chunk(0, 0, 0)  # qT mt0 n0 (first 512 qi)

            # Remaining work dripped into the attention loops, emitted between
            # a step's exp and its ctx matmuls so the PE work hides in the
            # exp's shadow. Phases are (c, mt) pairs in order:
            #   (0,0) (0,1) (1,0) (1,1) (2,0) (2,1) (3,0) (3,1)
            # mid[(c, mt, kj)] = list of thunks.
            mid = {}
            # (0,0): rest of kT mt0 (needed by its own kj>=4), the (0,1)
            # phase's kT mt1 n0 + qT mt1 n0, and all v chunks (jt=kj).
            mid[(0, 0, 1)] = [lambda: qk_chunk(1, 0, 1)]
            mid[(0, 0, 3)] = [lambda: qk_chunk(1, 0, 2)]
            mid[(0, 0, 5)] = [lambda: qk_chunk(1, 0, 3)]
            mid[(0, 0, 7)] = [lambda: qk_chunk(1, 1, 0)]
            mid[(0, 0, 9)] = [lambda: qk_chunk(0, 1, 0)]
            # (0,1): rest of kT mt1; qT chunks for phase (1,*)
            mid[(0, 1, 1)] = [lambda: qk_chunk(1, 1, 1)]
            mid[(0, 1, 3)] = [lambda: qk_chunk(1, 1, 2)]
            mid[(0, 1, 5)] = [lambda: qk_chunk(1, 1, 3)]
            mid[(0, 1, 7)] = [lambda: qk_chunk(0, 0, 1)]
            mid[(0, 1, 9)] = [lambda: qk_chunk(0, 1, 1)]
            # later qT chunks, one phase ahead of use
            mid[(1, 0, 1)] = [lambda: qk_chunk(0, 0, 2)]
            mid[(1, 1, 1)] = [lambda: qk_chunk(0, 1, 2)]
            mid[(2, 0, 1)] = [lambda: qk_chunk(0, 0, 3)]
            mid[(2, 1, 1)] = [lambda: qk_chunk(0, 1, 3)]
            # out-proj for chunk c drips into chunk c+1's phases
            for c in range(3):
                for i in range(8):
                    st, nt = i // 2, i % 2
                    mt_, kj_ = (0, 3 + 2 * (i % 4)) if i < 4 else (1, 3 + 2 * (i % 4))
                    mid.setdefault((c + 1, mt_, kj_), []).append(
                        lambda st=st, nt=nt, c=c: out_proj_piece(st, nt, c)
                    )

            NCH = S // 512  # 4 qi chunks of 512
            steps = [
                (c, mt, kj)
                for c in range(NCH)
                for mt in range(2)
                for kj in range(KJT)
            ]
            sc_t = {}

            def emit_sc(i):
                c, mt, kj = steps[i]
                sc = scp.tile([128, QC], f32, tag="sc", name=f"sc{c}{mt}{kj}")
                col = slice(c * 512, (c + 1) * 512)
                for hp in range(2):
                    hs = slice(64 * hp, 64 * hp + 64)
                    nc.tensor.matmul(
                        sc[:, hp * 512 : (hp + 1) * 512],
                        kT_sb[hs, mt, kj * 128 : (kj + 1) * 128],
                        qT_sb[hs, mt, col],
                        start=True,
                        stop=True,
                    )
                sc_t[i] = sc

            cxh = {}
            emit_sc(0)
            emit_sc(1)
            for i, (c, mt, kj) in enumerate(steps):
                col = slice(c * 512, (c + 1) * 512)
                if kj == 0:
                    cxh[(c, mt)] = [
                        cxp.tile(
                            [DK + 1, 512], f32, tag="cx", name=f"cx{c}{mt}{j}"
                        )
                        for j in range(2)
                    ]
                at = atp.tile([128, QC], f32r, tag="at")
                nc.scalar.activation(at[:], sc_t.pop(i)[:], Exp, scale=0.125)
                # scores two steps ahead, then background work, then ctx --
                # keeps the next exp's input first in PE program order so the
                # dripped matmuls hide in the exp shadow.
                if i + 2 < len(steps):
                    emit_sc(i + 2)
                for th in mid.get((c, mt, kj), ()):
                    th()
                if c == 0 and mt == 0:
                    v_chunk(kj)
                for hp in range(2):
                    nc.tensor.matmul(
                        cxh[(c, mt)][hp][:],
                        v_sb[:, kj, 2 * mt + hp, :],
                        at[:, hp * 512 : (hp + 1) * 512],
                        start=(kj == 0),
                        stop=(kj == KJT - 1),
                    )
                if kj == KJT - 1:
                    # normalize both heads: ctxT = cx[0:64] / cx[64].
                    # Copy psum->sbuf immediately (frees the psum bank), then
                    # run the whole chain in SBUF.
                    for hp in range(2):
                        cx = cxh[(c, mt)][hp]
                        cxs = npl.tile([DK + 1, 512], f32, tag="cxs")
                        nc.vector.tensor_copy(cxs[:], cx[:])
                        srow0 = npl.tile([1, 512], f32, tag="srow0")
                        nc.sync.dma_start(
                            out=srow0[:], in_=cxs[DK : DK + 1, :]
                        )
                        sbc = npl.tile([64, 512], f32, tag="sbc")
                        nc.gpsimd.partition_broadcast(sbc[:], srow0[:])
                        rinv = npl.tile([64, 512], f32, tag="rinv")
                        nc.vector.reciprocal_approx_fast(
                            out=rinv[:], in_=sbc[:]
                        )
                        if hp == 0:
                            nc.vector.tensor_mul(
                                ctxT_sb[0:64, mt, col], cxs[0:64, :], rinv[:]
                            )
                        else:
                            tmp = npl.tile([64, 512], f32r, tag="sbc")
                            nc.vector.tensor_mul(tmp[:], cxs[0:64, :], rinv[:])
                            nc.sync.dma_start(
                                out=ctxT_sb[64:128, mt, col], in_=tmp[:]
                            )
            # last chunk's out-proj is the unavoidable tail
            for st in range(4):
                for nt in range(2):
                    out_proj_piece(st, nt, NCH - 1, tail=True)

    nc.compile()
    return nc


def _get_nc():
    global _CACHED_NC
    if _CACHED_NC is None:
        _CACHED_NC = _build()
    return _CACHED_NC


def _in_maps(x, Wq, bq, Wk, bk, Wv, bv, Wo, bo):
    xTs = [np.ascontiguousarray(x[b].T) for b in range(B)]
    maps = []
    for c in range(N_CORES):
        b, g = c // 4, c % 4
        cs = slice(g * DL, (g + 1) * DL)
        maps.append(
            {
                "xT": xTs[b],
                "wq": np.ascontiguousarray(Wq[:, cs]),
                "wk": np.ascontiguousarray(Wk[:, cs]),
                "wv": np.ascontiguousarray(Wv[:, cs]),
                "wo": np.ascontiguousarray(Wo[cs, :]),
                "bq": np.ascontiguousarray(bq[cs].reshape(2, 128).T),
                "bk": np.ascontiguousarray(bk[cs].reshape(2, 128).T),
            }
        )
    return maps


def _assemble(results, bv, Wo, bo):
    corr = (bv.astype(np.float64) @ Wo.astype(np.float64)) + bo.astype(np.float64)
    outs = []
    for b in range(B):
        acc = np.zeros((S, D), dtype=np.float64)
        for g in range(4):
            acc += results[b * 4 + g]["out"].astype(np.float64)
        outs.append((acc + corr).astype(np.float32))
    return np.stack(outs)


def kernel(x, Wq, bq, Wk, bk, Wv, bv, Wo, bo):
    from concourse.bass_utils import run_bass_kernel_spmd

    x = np.asarray(x, dtype=np.float32)
    Wq = np.asarray(Wq, dtype=np.float32)
    Wk = np.asarray(Wk, dtype=np.float32)
    Wv = np.asarray(Wv, dtype=np.float32)
    Wo = np.asarray(Wo, dtype=np.float32)
    bq = np.asarray(bq, dtype=np.float32)
    bk = np.asarray(bk, dtype=np.float32)
    bv = np.asarray(bv, dtype=np.float32)
    bo = np.asarray(bo, dtype=np.float32)

    nc = _get_nc()
    res = run_bass_kernel_spmd(
        nc, _in_maps(x, Wq, bq, Wk, bk, Wv, bv, Wo, bo), core_ids=list(range(N_CORES))
    )
    return _assemble(res.results, bv, Wo, bo)



# revision 15
# speedup vs baseline: 1.1358x; 1.1358x over previous
"""MultiHeadAttention Trainium2 Bass kernel (8 cores).

Problem: B=2, S=2048, D=1024, H=16 heads, DK=64, fp32 in/out.
  q/k/v = x @ W* + b*; scores = q k^T / 8; attn = softmax; ctx = attn v;
  out = ctx @ Wo + bo.

Sharding (8 cores): batch (2-way) x head-group (4-way tensor parallel).
Core c handles b = c // 4 and heads [4g, 4g+4), g = c % 4 (d' slice of 256).
Each core computes a partial out [S, D] (contraction over its 256 d' rows of
Wo); the host sums the 4 partials per batch and adds the host-folded bias
correction (bv @ Wo + bo).

All on-device data is float16 (PE accumulates in fp32 PSUM; rel err ~1e-3,
budget 2e-2). f16 halves DMA bytes vs the old fp32r version and removes
fp32r's <256-free-size matmul penalty.

Layout ("transposed activations"): qT/kT [256, S] (d' on partitions, as
[128, 2(mt), S]), v natural [S, 256] stored per kj tile/head with an
augmented ones column for softmax denominators. Attention per
(qi-chunk c, head-pair mt, kj tile):
  scoresT[kj, qi] = kT^T qT          (PE)
  attnT = exp(scoresT / 8)           (ACT, psum->sbuf f16)
  cx[.., qi]    += v_aug^T attnT     (PE psum accumulate over kj)
where cx is ONE [128, 1024] psum tile per (c, mt):
  cx[0:64,   0:512]    = ctx head 2mt   (v_aug = [v | 1], sum row at 64)
  cx[64:65,  0:512]    = denom head 2mt
  cx[63:64,  512:1024] = denom head 2mt+1
  cx[64:128, 512:1024] = ctx head 2mt+1 (v_aug = [1 | v], sum row first)
This puts each head's ctx rows at their FINAL ctxT partitions (0-63 /
64-127), so normalization needs no partition-shifting DMA: copy cx->sbuf
(frees psum), reciprocal of the two denom rows (DVE, same partitions),
broadcast via PE rank-1 outer product (ones[1,64]^T x row), then two
same-partition DVE muls into ctxT.

out-proj: out[s,:] = sum_mt ctxT[:, mt, s]^T wo[mt] (PE), copy f16, DMA.

The projection matmuls (qT/kT/v) and out-proj are dripped into the
attention loop in fine-grained units sized so PE never idles (PE is the
bottleneck engine: ~164us of matmul rows at 2.4 GHz vs ~136us ACT exp).
Per-iter PE program order: [drips] [PE norm outers] [sc(i+2)] [ctx(i)] --
drips first so the exp(i)/slot-free waits are covered by queued PE work.

DMA: x is host-packed f16 [128, 4(n), 8(kt), 512] and streamed per
n-block (qi/s column blocks) in the order the drip schedule consumes it;
weights split per mt half so the first matmuls start ~3us in.
"""

import numpy as np

B = 2
S = 2048
D = 1024
H = 16
DK = 64
N_CORES = 8
HL = H // 4  # 4 heads per core
DL = HL * DK  # 256 local d'
KJT = S // 128  # 16 kj tiles
KT = D // 128  # 8 contraction tiles
NCH = S // 512  # 4 qi chunks

_CACHED_NC = None


def _build():
    import concourse.bacc as bacc
    import concourse.mybir as mybir
    import concourse.tile as tile

    f32 = mybir.dt.float32
    f16 = mybir.dt.float16
    Exp = mybir.ActivationFunctionType.Exp

    nc = bacc.Bacc(None)

    xp = nc.declare_dram_parameter("xp", [128, NCH, KT, 512], f16, isOutput=False)
    wq0 = nc.declare_dram_parameter("wq0", [128, KT, 128], f16, isOutput=False)
    wq1 = nc.declare_dram_parameter("wq1", [128, KT, 128], f16, isOutput=False)
    wk0 = nc.declare_dram_parameter("wk0", [128, KT, 128], f16, isOutput=False)
    wk1 = nc.declare_dram_parameter("wk1", [128, KT, 128], f16, isOutput=False)
    wv0 = nc.declare_dram_parameter("wv0", [128, KT, 128], f16, isOutput=False)
    wv1 = nc.declare_dram_parameter("wv1", [128, KT, 128], f16, isOutput=False)
    wo = nc.declare_dram_parameter("wo", [128, 2, D], f16, isOutput=False)
    bq = nc.declare_dram_parameter("bq", [128, 2], f32, isOutput=False)
    bk = nc.declare_dram_parameter("bk", [128, 2], f32, isOutput=False)
    out = nc.declare_dram_parameter("out", [S, D], f16, isOutput=True)

    with tile.TileContext(nc) as tc:
        with (
            tc.tile_pool(name="persist", bufs=1) as persist,
            tc.tile_pool(name="attn", bufs=3) as atp,
            tc.tile_pool(name="norm", bufs=2) as npl,
            tc.tile_pool(name="ob", bufs=2) as obp,
            tc.tile_pool(name="scps", bufs=2, space="PSUM") as scp,
            tc.tile_pool(name="cxps", bufs=1, space="PSUM") as cxp,
            tc.tile_pool(name="wsps", bufs=2, space="PSUM") as wsp,
        ):
            xall = persist.tile([128, NCH, KT, 512], f16, tag="xall")
            wqs = persist.tile([128, 2, KT, 128], f16, tag="wqs")
            wks = persist.tile([128, 2, KT, 128], f16, tag="wks")
            wvs = persist.tile([128, 2, KT, 128], f16, tag="wvs")
            wo_sb = persist.tile([128, 2, D], f16, tag="wo")
            qT_sb = persist.tile([128, 2, S], f16, tag="qT")
            kT_sb = persist.tile([128, 2, S], f16, tag="kT")
            v_sb = persist.tile([128, KJT, HL, DK + 1], f16, tag="v")
            ctxT_sb = persist.tile([128, 2, S], f16, tag="ctxT")
            ones_sb = persist.tile([128, 64], f16, tag="ones")
            bq_sb = persist.tile([128, 2], f32, tag="bq")
            bk_sb = persist.tile([128, 2], f32, tag="bk")

            # --- input DMAs, in consumption order ---
            nc.sync.dma_start(out=wks[:, 0], in_=wk0[:])
            nc.sync.dma_start(out=xall[:, 0], in_=xp[:, 0])
            nc.sync.dma_start(out=wqs[:, 0], in_=wq0[:])
            nc.sync.dma_start(out=bq_sb[:], in_=bq[:])
            nc.sync.dma_start(out=bk_sb[:], in_=bk[:])
            nc.sync.dma_start(out=wvs[:, 0], in_=wv0[:])
            nc.sync.dma_start(out=xall[:, 1], in_=xp[:, 1])
            nc.sync.dma_start(out=xall[:, 2], in_=xp[:, 2])
            nc.sync.dma_start(out=wks[:, 1], in_=wk1[:])
            nc.sync.dma_start(out=wqs[:, 1], in_=wq1[:])
            nc.sync.dma_start(out=wvs[:, 1], in_=wv1[:])
            nc.sync.dma_start(out=xall[:, 3], in_=xp[:, 3])
            nc.sync.dma_start(out=wo_sb[:], in_=wo[:])

            nc.vector.memset(ones_sb[:], 1.0)
            nc.vector.memset(v_sb[:, :, :, DK : DK + 1], 1.0)  # [v | 1]

            # --- projection chunk emitters (drip units) ---
            def qk_unit(which, mt, n, u, state={}):
                """Half a qT/kT [128, 512] chunk: 4 matmuls; u=1 finalizes."""
                wt, dst, bias = (
                    (wqs, qT_sb, bq_sb) if which == 0 else (wks, kT_sb, bk_sb)
                )
                key = (which, mt, n)
                if u == 0:
                    state[key] = wsp.tile(
                        [128, 512], f32, tag="ws", name=f"pj{which}{mt}{n}"
                    )
                ps = state[key]
                for kt in range(4 * u, 4 * u + 4):
                    nc.tensor.matmul(
                        ps[:],
                        wt[:, mt, kt, :],
                        xall[:, n, kt, :],
                        start=(kt == 0),
                        stop=(kt == KT - 1),
                    )
                if u == 1:
                    ns = slice(n * 512, (n + 1) * 512)
                    nc.vector.tensor_scalar_add(
                        out=dst[:, mt, ns],
                        in0=ps[:],
                        scalar1=bias[:, mt : mt + 1],
                    )
                    del state[key]

            def v_pair(jt, mt):
                """v rows [jt*128,(jt+1)*128) for head pair (2mt, 2mt+1)."""
                n, so = jt // 4, (jt % 4) * 128
                ps = wsp.tile([128, 128], f32, tag="ws", name=f"vp{jt}{mt}")
                for kt in range(KT):
                    nc.tensor.matmul(
                        ps[:],
                        xall[:, n, kt, so : so + 128],
                        wvs[:, mt, kt, :],
                        start=(kt == 0),
                        stop=(kt == KT - 1),
                    )
                nc.vector.tensor_copy(
                    v_sb[:, jt, 2 * mt : 2 * mt + 2, 0:DK],
                    ps[:, 0:128].rearrange("p (h d) -> p h d", h=2),
                )

            ob_state = {}

            def out_piece(c, st, nt, tail=False):
                """Partial out rows [c*512+st*128, +128), cols [nt*512, +512)."""
                s0 = c * 512 + st * 128
                op = wsp.tile([128, 512], f32, tag="ws", name=f"op{c}{st}{nt}")
                for mt2 in range(2):
                    nc.tensor.matmul(
                        op[:],
                        ctxT_sb[:, mt2, s0 : s0 + 128],
                        wo_sb[:, mt2, nt * 512 : (nt + 1) * 512],
                        start=(mt2 == 0),
                        stop=(mt2 == 1),
                    )
                if nt == 0:
                    ob_state[(c, st)] = obp.tile(
                        [128, D], f16, tag="ob", name=f"ob{c}{st}"
                    )
                ob = ob_state[(c, st)]
                if tail and nt == 0:
                    nc.scalar.copy(ob[:, nt * 512 : (nt + 1) * 512], op[:])
                else:
                    nc.vector.tensor_copy(ob[:, nt * 512 : (nt + 1) * 512], op[:])
                if nt == 1:
                    nc.sync.dma_start(out=out[s0 : s0 + 128, :], in_=ob[:])
                    del ob_state[(c, st)]

            # --- drip schedule: iter index -> list of thunks ---
            drip = {}

            def add(i, fn, *a, **k):
                drip.setdefault(i, []).append(lambda: fn(*a, **k))

            for j in range(KJT):  # v pairs: phase 0 -> mt0, phase 1 -> mt1
                add(j, v_pair, j, 0)
                add(16 + j, v_pair, j, 1)
            # remaining kT chunks + next-phase prologues
            add(1, qk_unit, 1, 0, 1, 0)
            add(2, qk_unit, 1, 0, 1, 1)
            add(4, qk_unit, 1, 0, 2, 0)
            add(5, qk_unit, 1, 0, 2, 1)
            add(6, qk_unit, 1, 0, 3, 0)
            add(7, qk_unit, 1, 0, 3, 1)
            add(8, qk_unit, 1, 1, 0, 0)
            add(9, qk_unit, 1, 1, 0, 1)
            add(10, qk_unit, 0, 1, 0, 0)
            add(11, qk_unit, 0, 1, 0, 1)
            add(16, qk_unit, 1, 1, 1, 0)
            add(16, qk_unit, 1, 1, 1, 1)
            add(17, qk_unit, 1, 1, 2, 0)
            add(18, qk_unit, 1, 1, 2, 1)
            add(19, qk_unit, 1, 1, 3, 0)
            add(20, qk_unit, 1, 1, 3, 1)
            add(21, qk_unit, 0, 0, 1, 0)
            add(22, qk_unit, 0, 0, 1, 1)
            add(32, qk_unit, 0, 1, 1, 0)
            add(32, qk_unit, 0, 1, 1, 1)
            add(33, qk_unit, 0, 0, 2, 0)
            add(34, qk_unit, 0, 0, 2, 1)
            add(48, qk_unit, 0, 1, 2, 0)
            add(48, qk_unit, 0, 1, 2, 1)
            add(64, qk_unit, 0, 0, 3, 0)
            add(64, qk_unit, 0, 0, 3, 1)
            add(80, qk_unit, 0, 1, 3, 0)
            add(80, qk_unit, 0, 1, 3, 1)
            # out-proj drips: out(c) after phase (c,1) ends
            for p, (st, nt) in enumerate((s, n) for s in range(4) for n in range(2)):
                add(35 + p, out_piece, 0, st, nt)  # its 35-42
                add(65 + p, out_piece, 1, st, nt)  # its 65-72
                add(97 + p if p < 6 else 112 + (p - 6), out_piece, 2, st, nt)

            # --- attention spine ---
            steps = [
                (c, mt, kj)
                for c in range(NCH)
                for mt in range(2)
                for kj in range(KJT)
            ]
            sc_t = {}

            def emit_sc(i):
                c, mt, kj = steps[i]
                sc = scp.tile([128, 1024], f32, tag="sc", name=f"sc{c}{mt}{kj}")
                col = slice(c * 512, (c + 1) * 512)
                for hp in range(2):
                    hs = slice(64 * hp, 64 * hp + 64)
                    nc.tensor.matmul(
                        sc[:, hp * 512 : (hp + 1) * 512],
                        kT_sb[hs, mt, kj * 128 : (kj + 1) * 128],
                        qT_sb[hs, mt, col],
                        start=True,
                        stop=True,
                    )
                sc_t[i] = sc

            def norm_pre(c, mt, cxt, use_act):
                """Copy cx psum -> sbuf (frees the bank) + denom reciprocals.

                cx layout: [0:64, 0:512] ctx head 2mt, [0:64, 512:1024] ctx
                head 2mt+1, row 64 = both denominators.
                """
                cxs = npl.tile([128, 1024], f32, tag="cxs")
                if use_act:
                    nc.scalar.copy(cxs[0:65, :], cxt[0:65, :])
                else:
                    nc.vector.tensor_copy(cxs[0:65, :], cxt[0:65, :])
                rr = npl.tile([128, 1024], f32, tag="rr")
                rt = npl.tile([128, 1024], f32, tag="rt")
                # denom row lives on partition 64; custom-DVE recip can't
                # cross quadrants, so plain-copy it to partition 0 first.
                nc.vector.tensor_copy(rt[0:1, :], cxs[64:65, :])
                nc.vector.reciprocal_approx_fast(out=rr[0:1, :], in_=rt[0:1, :])
                return cxs, rr

            def norm_bcast(cxs, rr, use_pe):
                """Broadcast recip row to 64 partitions; returns (apA, apB)."""
                bc = npl.tile([128, 1024], f32, tag="bcs")
                nc.gpsimd.partition_broadcast(bc[0:64, :], rr[0:1, :])
                return bc[:, 0:512], bc[:, 512:1024]

            def norm_mul(c, mt, cxs, bca, bcb):
                col = slice(c * 512, (c + 1) * 512)
                nc.vector.tensor_mul(
                    ctxT_sb[0:64, mt, col], cxs[0:64, 0:512], bca[0:64, :]
                )
                # head 2mt+1 lands on partitions 64-127: <=32-channel DVE ops
                # may write cross-quadrant, so split into two 32-row ops.
                for q in range(2):
                    qs = slice(32 * q, 32 * q + 32)
                    nc.vector.tensor_mul(
                        ctxT_sb[64 + 32 * q : 96 + 32 * q, mt, col],
                        cxs[qs, 512:1024],
                        bcb[qs, :],
                    )

            # prologue: kT(mt0,n0) + qT(mt0,n0), then 2 sc tiles ahead
            qk_unit(1, 0, 0, 0)
            qk_unit(1, 0, 0, 1)
            qk_unit(0, 0, 0, 0)
            qk_unit(0, 0, 0, 1)
            emit_sc(0)
            emit_sc(1)

            cx_t = {}
            pend_norm = None
            for i, (c, mt, kj) in enumerate(steps):
                if kj == 0:
                    cx_t[(c, mt)] = cxp.tile(
                        [128, 1024], f32, tag="cx", name=f"cx{c}{mt}"
                    )
                at = atp.tile([128, 1024], f16, tag="at")
                nc.scalar.activation(at[:], sc_t.pop(i)[:], Exp, scale=0.125)
                nb = None
                if kj == 0 and i > 0:
                    pc, pmt = steps[i - 1][0], steps[i - 1][1]
                    cxs, rr = norm_pre(pc, pmt, cx_t.pop((pc, pmt)), i <= 32)
                    nb = (pc, pmt, cxs, rr)
                for th in drip.get(i, ()):
                    th()
                if nb is not None:
                    bca, bcb = norm_bcast(nb[2], nb[3], use_pe=False)
                if i + 2 < len(steps):
                    emit_sc(i + 2)
                if nb is not None:
                    norm_mul(nb[0], nb[1], nb[2], bca, bcb)
                cxt = cx_t[(c, mt)]
                for hp in range(2):
                    nc.tensor.matmul(
                        cxt[0:65, hp * 512 : (hp + 1) * 512],
                        v_sb[:, kj, 2 * mt + hp, :],
                        at[:, hp * 512 : (hp + 1) * 512],
                        start=(kj == 0),
                        stop=(kj == KJT - 1),
                    )

            # tail: last phase's normalize + final out-proj chunk
            cxs, rr = norm_pre(NCH - 1, 1, cx_t.pop((NCH - 1, 1)), False)
            bca, bcb = norm_bcast(cxs, rr, use_pe=True)
            norm_mul(NCH - 1, 1, cxs, bca, bcb)
            for st in range(4):
                for nt in range(2):
                    out_piece(3, st, nt, tail=True)

    nc.compile()
    return nc


def _get_nc():
    global _CACHED_NC
    if _CACHED_NC is None:
        _CACHED_NC = _build()
    return _CACHED_NC


def _pack_w_half(W, g, mt):
    """[128, 8, 128] f16: [p, kt, col] = W[kt*128+p, g*256+mt*128+col]."""
    sl = W[:, g * DL + mt * 128 : g * DL + (mt + 1) * 128]
    return np.ascontiguousarray(
        sl.reshape(KT, 128, 128).transpose(1, 0, 2).astype(np.float16)
    )


def _in_maps(x, Wq, bq, Wk, bk, Wv, bv, Wo, bo):
    xpacks = []
    for b in range(B):
        xpacks.append(
            np.ascontiguousarray(
                x[b]
                .reshape(NCH, 512, KT, 128)
                .transpose(3, 0, 2, 1)
                .astype(np.float16)
            )
        )
    maps = []
    for c in range(N_CORES):
        b, g = c // 4, c % 4
        cs = slice(g * DL, (g + 1) * DL)
        maps.append(
            {
                "xp": xpacks[b],
                "wq0": _pack_w_half(Wq, g, 0),
                "wq1": _pack_w_half(Wq, g, 1),
                "wk0": _pack_w_half(Wk, g, 0),
                "wk1": _pack_w_half(Wk, g, 1),
                "wv0": _pack_w_half(Wv, g, 0),
                "wv1": _pack_w_half(Wv, g, 1),
                "wo": np.ascontiguousarray(
                    Wo[cs, :]
                    .reshape(2, 128, D)
                    .transpose(1, 0, 2)
                    .astype(np.float16)
                ),
                "bq": np.ascontiguousarray(bq[cs].reshape(2, 128).T.astype(np.float32)),
                "bk": np.ascontiguousarray(bk[cs].reshape(2, 128).T.astype(np.float32)),
            }
        )
    return maps


def _assemble(results, bv, Wo, bo):
    corr = (bv.astype(np.float64) @ Wo.astype(np.float64)) + bo.astype(np.float64)
    outs = []
    for b in range(B):
        acc = np.zeros((S, D), dtype=np.float64)
        for g in range(4):
            acc += results[b * 4 + g]["out"].astype(np.float64)
        outs.append((acc + corr).astype(np.float32))
    return np.stack(outs)


def kernel(x, Wq, bq, Wk, bk, Wv, bv, Wo, bo):
    from concourse.bass_utils import run_bass_kernel_spmd

    x = np.asarray(x, dtype=np.float32)
    Wq = np.asarray(Wq, dtype=np.float32)
    Wk = np.asarray(Wk, dtype=np.float32)
    Wv = np.asarray(Wv, dtype=np.float32)
    Wo = np.asarray(Wo, dtype=np.float32)
    bq = np.asarray(bq, dtype=np.float32)
    bk = np.asarray(bk, dtype=np.float32)
    bv = np.asarray(bv, dtype=np.float32)
    bo = np.asarray(bo, dtype=np.float32)

    nc = _get_nc()
    res = run_bass_kernel_spmd(
        nc, _in_maps(x, Wq, bq, Wk, bk, Wv, bv, Wo, bo), core_ids=list(range(N_CORES))
    )
    return _assemble(res.results, bv, Wo, bo)


# revision 21
# speedup vs baseline: 1.1832x; 1.0417x over previous
"""MultiHeadAttention Trainium2 Bass kernel (8 cores).

Problem: B=2, S=2048, D=1024, H=16 heads, DK=64, fp32 in/out.
  q/k/v = x @ W* + b*; scores = q k^T / 8; attn = softmax; ctx = attn v;
  out = ctx @ Wo + bo.

Sharding (8 cores): batch (2-way) x head-group (4-way tensor parallel).
Core c handles b = c // 4 and heads [4g, 4g+4), g = c % 4 (d' slice of 256).
Each core computes a partial out [S, D] (contraction over its 256 d' rows of
Wo); the host sums the 4 partials per batch and adds the host-folded bias
correction (bv @ Wo + bo).

All on-device data is float16 (PE accumulates in fp32 PSUM; rel err ~1e-3,
budget 2e-2). f16 halves DMA bytes vs the old fp32r version and removes
fp32r's <256-free-size matmul penalty.

Layout ("transposed activations"): qT/kT [256, S] (d' on partitions, as
[128, 2(mt), S]), v natural [S, 256] stored per kj tile/head with an
augmented ones column for softmax denominators. Attention per
(qi-chunk c, head-pair mt, kj tile):
  scoresT[kj, qi] = kT^T qT          (PE)
  attnT = exp(scoresT / 8)           (ACT, psum->sbuf f16)
  cx[.., qi]    += v_aug^T attnT     (PE psum accumulate over kj)
where cx is ONE [128, 1024] psum tile per (c, mt):
  cx[0:64,   0:512]    = ctx head 2mt   (v_aug = [v | 1], sum row at 64)
  cx[64:65,  0:512]    = denom head 2mt
  cx[63:64,  512:1024] = denom head 2mt+1
  cx[64:128, 512:1024] = ctx head 2mt+1 (v_aug = [1 | v], sum row first)
This puts each head's ctx rows at their FINAL ctxT partitions (0-63 /
64-127), so normalization needs no partition-shifting DMA: copy cx->sbuf
(frees psum), reciprocal of the two denom rows (DVE, same partitions),
broadcast via PE rank-1 outer product (ones[1,64]^T x row), then two
same-partition DVE muls into ctxT.

out-proj: out[s,:] = sum_mt ctxT[:, mt, s]^T wo[mt] (PE), copy f16, DMA.

The projection matmuls (qT/kT/v) and out-proj are dripped into the
attention loop in fine-grained units sized so PE never idles (PE is the
bottleneck engine: ~164us of matmul rows at 2.4 GHz vs ~136us ACT exp).
Per-iter PE program order: [drips] [PE norm outers] [sc(i+2)] [ctx(i)] --
drips first so the exp(i)/slot-free waits are covered by queued PE work.

DMA: x is host-packed f16 [128, 4(n), 8(kt), 512] and streamed per
n-block (qi/s column blocks) in the order the drip schedule consumes it;
weights split per mt half so the first matmuls start ~3us in.
"""

import numpy as np

B = 2
S = 2048
D = 1024
H = 16
DK = 64
N_CORES = 8
HL = H // 4  # 4 heads per core
DL = HL * DK  # 256 local d'
KJT = S // 128  # 16 kj tiles
KT = D // 128  # 8 contraction tiles
NCH = S // 512  # 4 qi chunks

_CACHED_NC = None


def _build():
    import concourse.bacc as bacc
    import concourse.mybir as mybir
    import concourse.tile as tile

    f32 = mybir.dt.float32
    f16 = mybir.dt.float16
    Exp = mybir.ActivationFunctionType.Exp

    nc = bacc.Bacc(None)

    xp = nc.declare_dram_parameter("xp", [128, NCH, KT, 512], f16, isOutput=False)
    wq0 = nc.declare_dram_parameter("wq0", [128, KT, 128], f16, isOutput=False)
    wq1 = nc.declare_dram_parameter("wq1", [128, KT, 128], f16, isOutput=False)
    wk0 = nc.declare_dram_parameter("wk0", [128, KT, 128], f16, isOutput=False)
    wk1 = nc.declare_dram_parameter("wk1", [128, KT, 128], f16, isOutput=False)
    wv0 = nc.declare_dram_parameter("wv0", [128, KT, 128], f16, isOutput=False)
    wv1 = nc.declare_dram_parameter("wv1", [128, KT, 128], f16, isOutput=False)
    wo = nc.declare_dram_parameter("wo", [128, 2, D], f16, isOutput=False)
    bq = nc.declare_dram_parameter("bq", [128, 2], f32, isOutput=False)
    bk = nc.declare_dram_parameter("bk", [128, 2], f32, isOutput=False)
    out = nc.declare_dram_parameter("out", [S, D], f16, isOutput=True)

    with tile.TileContext(nc) as tc:
        with (
            tc.tile_pool(name="persist", bufs=1) as persist,
            tc.tile_pool(name="attn", bufs=3) as atp,
            tc.tile_pool(name="norm", bufs=2) as npl,
            tc.tile_pool(name="ob", bufs=2) as obp,
            tc.tile_pool(name="scps", bufs=2, space="PSUM") as scp,
            tc.tile_pool(name="cxps", bufs=1, space="PSUM") as cxp,
            tc.tile_pool(name="wsps", bufs=2, space="PSUM") as wsp,
        ):
            xall = persist.tile([128, NCH, KT, 512], f16, tag="xall")
            wqs = persist.tile([128, 2, KT, 128], f16, tag="wqs")
            wks = persist.tile([128, 2, KT, 128], f16, tag="wks")
            wvs = persist.tile([128, 2, KT, 128], f16, tag="wvs")
            wo_sb = persist.tile([128, 2, D], f16, tag="wo")
            qT_sb = persist.tile([128, 2, S], f16, tag="qT")
            kT_sb = persist.tile([128, 2, S], f16, tag="kT")
            v_sb = persist.tile([128, KJT, HL, DK + 1], f16, tag="v")
            ctxT_sb = persist.tile([128, 2, S], f16, tag="ctxT")
            ones_sb = persist.tile([128, 64], f16, tag="ones")
            bq_sb = persist.tile([128, 2], f32, tag="bq")
            bk_sb = persist.tile([128, 2], f32, tag="bk")

            # --- input DMAs, in consumption order; x n0 split in half so the
            # first narrow kT/qT chunks start ~2us earlier ---
            nc.sync.dma_start(out=wks[:, 0], in_=wk0[:])
            nc.sync.dma_start(out=xall[:, 0, :, 0:256], in_=xp[:, 0, :, 0:256])
            nc.sync.dma_start(out=wqs[:, 0], in_=wq0[:])
            nc.sync.dma_start(out=xall[:, 0, :, 256:512], in_=xp[:, 0, :, 256:512])
            nc.sync.dma_start(out=bq_sb[:], in_=bq[:])
            nc.sync.dma_start(out=bk_sb[:], in_=bk[:])
            nc.sync.dma_start(out=wvs[:, 0], in_=wv0[:])
            nc.sync.dma_start(out=xall[:, 1], in_=xp[:, 1])
            nc.sync.dma_start(out=xall[:, 2], in_=xp[:, 2])
            nc.sync.dma_start(out=wks[:, 1], in_=wk1[:])
            nc.sync.dma_start(out=wqs[:, 1], in_=wq1[:])
            nc.sync.dma_start(out=wvs[:, 1], in_=wv1[:])
            nc.sync.dma_start(out=xall[:, 3], in_=xp[:, 3])
            nc.sync.dma_start(out=wo_sb[:], in_=wo[:])

            nc.vector.memset(ones_sb[:], 1.0)
            nc.vector.memset(v_sb[:, :, :, DK : DK + 1], 1.0)  # [v | 1]

            # --- projection chunk emitters (drip units) ---
            def qk_unit(which, mt, n, u, state={}):
                """Half a qT/kT [128, 512] chunk: 4 matmuls; u=1 finalizes."""
                wt, dst, bias = (
                    (wqs, qT_sb, bq_sb) if which == 0 else (wks, kT_sb, bk_sb)
                )
                key = (which, mt, n)
                if u == 0:
                    state[key] = wsp.tile(
                        [128, 512], f32, tag="ws", name=f"pj{which}{mt}{n}"
                    )
                ps = state[key]
                for kt in range(4 * u, 4 * u + 4):
                    nc.tensor.matmul(
                        ps[:],
                        wt[:, mt, kt, :],
                        xall[:, n, kt, :],
                        start=(kt == 0),
                        stop=(kt == KT - 1),
                    )
                if u == 1:
                    ns = slice(n * 512, (n + 1) * 512)
                    nc.vector.tensor_scalar_add(
                        out=dst[:, mt, ns],
                        in0=ps[:],
                        scalar1=bias[:, mt : mt + 1],
                    )
                    del state[key]

            def v_pair(jt, mt):
                """v rows [jt*128,(jt+1)*128) for head pair (2mt, 2mt+1)."""
                n, so = jt // 4, (jt % 4) * 128
                ps = wsp.tile([128, 128], f32, tag="ws", name=f"vp{jt}{mt}")
                for kt in range(KT):
                    nc.tensor.matmul(
                        ps[:],
                        xall[:, n, kt, so : so + 128],
                        wvs[:, mt, kt, :],
                        start=(kt == 0),
                        stop=(kt == KT - 1),
                    )
                nc.vector.tensor_copy(
                    v_sb[:, jt, 2 * mt : 2 * mt + 2, 0:DK],
                    ps[:, 0:128].rearrange("p (h d) -> p h d", h=2),
                )

            ob_state = {}

            def out_piece(c, st, nt, tail=False):
                """Partial out rows [c*512+st*128, +128), cols [nt*512, +512)."""
                s0 = c * 512 + st * 128
                op = wsp.tile([128, 512], f32, tag="ws", name=f"op{c}{st}{nt}")
                for mt2 in range(2):
                    nc.tensor.matmul(
                        op[:],
                        ctxT_sb[:, mt2, s0 : s0 + 128],
                        wo_sb[:, mt2, nt * 512 : (nt + 1) * 512],
                        start=(mt2 == 0),
                        stop=(mt2 == 1),
                    )
                if nt == 0:
                    ob_state[(c, st)] = obp.tile(
                        [128, D], f16, tag="ob", name=f"ob{c}{st}"
                    )
                ob = ob_state[(c, st)]
                if tail and nt == 0:
                    nc.scalar.copy(ob[:, nt * 512 : (nt + 1) * 512], op[:])
                else:
                    nc.vector.tensor_copy(ob[:, nt * 512 : (nt + 1) * 512], op[:])
                if nt == 1:
                    nc.sync.dma_start(out=out[s0 : s0 + 128, :], in_=ob[:])
                    del ob_state[(c, st)]

            def qk_narrow(which, c0):
                """256-wide piece of the (which, mt0, n0) projection chunk."""
                wt, dst, bias = (
                    (wqs, qT_sb, bq_sb) if which == 0 else (wks, kT_sb, bk_sb)
                )
                ps = wsp.tile([128, 256], f32, tag="ws", name=f"nw{which}{c0}")
                for kt in range(KT):
                    nc.tensor.matmul(
                        ps[:],
                        wt[:, 0, kt, :],
                        xall[:, 0, kt, c0 : c0 + 256],
                        start=(kt == 0),
                        stop=(kt == KT - 1),
                    )
                nc.vector.tensor_scalar_add(
                    out=dst[:, 0, c0 : c0 + 256], in0=ps[:], scalar1=bias[:, 0:1]
                )

            # --- drip schedule: iter index -> list of thunks ---
            drip = {}

            def add(i, fn, *a, **k):
                drip.setdefault(i, []).append(lambda: fn(*a, **k))

            # kT(mt0) cols 256:512 first at iter 0 (only needs x n0 + wk0,
            # unlike v_pair which also waits on the wv DMA)
            add(0, qk_narrow, 1, 256)
            for j in range(KJT):  # v pairs: phase 0 -> mt0, phase 1 -> mt1
                add(j, v_pair, j, 0)
                add(16 + j, v_pair, j, 1)
            # remaining kT chunks + next-phase prologues
            add(1, qk_unit, 1, 0, 1, 0)
            add(2, qk_unit, 1, 0, 1, 1)
            add(4, qk_unit, 1, 0, 2, 0)
            add(5, qk_unit, 1, 0, 2, 1)
            add(6, qk_unit, 1, 0, 3, 0)
            add(7, qk_unit, 1, 0, 3, 1)
            add(8, qk_unit, 1, 1, 0, 0)
            add(9, qk_unit, 1, 1, 0, 1)
            add(10, qk_unit, 0, 1, 0, 0)
            add(11, qk_unit, 0, 1, 0, 1)
            add(16, qk_unit, 1, 1, 1, 0)
            add(16, qk_unit, 1, 1, 1, 1)
            add(17, qk_unit, 1, 1, 2, 0)
            add(18, qk_unit, 1, 1, 2, 1)
            add(19, qk_unit, 1, 1, 3, 0)
            add(20, qk_unit, 1, 1, 3, 1)
            add(21, qk_unit, 0, 0, 1, 0)
            add(22, qk_unit, 0, 0, 1, 1)
            add(32, qk_unit, 0, 1, 1, 0)
            add(32, qk_unit, 0, 1, 1, 1)
            add(33, qk_unit, 0, 0, 2, 0)
            add(34, qk_unit, 0, 0, 2, 1)
            add(48, qk_unit, 0, 1, 2, 0)
            add(48, qk_unit, 0, 1, 2, 1)
            add(64, qk_unit, 0, 0, 3, 0)
            add(64, qk_unit, 0, 0, 3, 1)
            add(80, qk_unit, 0, 1, 3, 0)
            add(80, qk_unit, 0, 1, 3, 1)
            # out-proj drips: out(c) available after phase (c,1); spread so
            # every late phase carries ~4 pieces (keeps PE ~= ACT per phase)
            for p, (st, nt) in enumerate((s, n) for s in range(4) for n in range(2)):
                add(36 + p if p < 4 else 49 + (p - 4), out_piece, 0, st, nt)
                add(66 + p if p < 4 else 81 + (p - 4), out_piece, 1, st, nt)
                add(97 + p if p < 4 else 112 + (p - 4), out_piece, 2, st, nt)

            # --- attention spine ---
            steps = [
                (c, mt, kj)
                for c in range(NCH)
                for mt in range(2)
                for kj in range(KJT)
            ]
            sc_t = {}

            def emit_sc(i):
                c, mt, kj = steps[i]
                sc = scp.tile([128, 1024], f32, tag="sc", name=f"sc{c}{mt}{kj}")
                col = slice(c * 512, (c + 1) * 512)
                for hp in range(2):
                    hs = slice(64 * hp, 64 * hp + 64)
                    nc.tensor.matmul(
                        sc[:, hp * 512 : (hp + 1) * 512],
                        kT_sb[hs, mt, kj * 128 : (kj + 1) * 128],
                        qT_sb[hs, mt, col],
                        start=True,
                        stop=True,
                    )
                sc_t[i] = sc

            def norm_pre(c, mt, cxt, use_act):
                """Copy cx psum -> sbuf (frees the bank) + denom reciprocals.

                cx layout: [0:64, 0:512] ctx head 2mt, [0:64, 512:1024] ctx
                head 2mt+1, row 64 = both denominators.
                """
                cxs = npl.tile([128, 1024], f32, tag="cxs")
                if use_act:
                    nc.scalar.copy(cxs[0:65, :], cxt[0:65, :])
                else:
                    nc.vector.tensor_copy(cxs[0:65, :], cxt[0:65, :])
                rr = npl.tile([128, 1024], f32, tag="rr")
                rt = npl.tile([128, 1024], f32, tag="rt")
                # denom row lives on partition 64; custom-DVE recip can't
                # cross quadrants, so plain-copy it to partition 0 first.
                nc.vector.tensor_copy(rt[0:1, :], cxs[64:65, :])
                nc.vector.reciprocal_approx_fast(out=rr[0:1, :], in_=rt[0:1, :])
                return cxs, rr

            def norm_bcast(cxs, rr, use_pe):
                """Broadcast recip row to 64 partitions; returns (apA, apB)."""
                bc = npl.tile([128, 1024], f32, tag="bcs")
                nc.gpsimd.partition_broadcast(bc[0:64, :], rr[0:1, :])
                return bc[:, 0:512], bc[:, 512:1024]

            def norm_mul(c, mt, cxs, bca, bcb):
                col = slice(c * 512, (c + 1) * 512)
                nc.vector.tensor_mul(
                    ctxT_sb[0:64, mt, col], cxs[0:64, 0:512], bca[0:64, :]
                )
                # head 2mt+1 lands on partitions 64-127: <=32-channel DVE ops
                # may write cross-quadrant, so split into two 32-row ops.
                for q in range(2):
                    qs = slice(32 * q, 32 * q + 32)
                    nc.vector.tensor_mul(
                        ctxT_sb[64 + 32 * q : 96 + 32 * q, mt, col],
                        cxs[qs, 512:1024],
                        bcb[qs, :],
                    )

            # prologue (narrow-start): kT cols 0:256 (covers kj 0-1), then
            # qT n0 in two halves; kT cols 256:512 drips at iter 0
            qk_narrow(1, 0)
            qk_narrow(0, 0)
            qk_narrow(0, 256)
            emit_sc(0)
            emit_sc(1)

            cx_t = {}
            pend_norm = None
            for i, (c, mt, kj) in enumerate(steps):
                if kj == 0:
                    cx_t[(c, mt)] = cxp.tile(
                        [128, 1024], f32, tag="cx", name=f"cx{c}{mt}"
                    )
                at = atp.tile([128, 1024], f16, tag="at")
                nc.scalar.activation(at[:], sc_t.pop(i)[:], Exp, scale=0.125)
                nb = None
                if kj == 0 and i > 0:
                    pc, pmt = steps[i - 1][0], steps[i - 1][1]
                    cxs, rr = norm_pre(pc, pmt, cx_t.pop((pc, pmt)), i <= 32)
                    nb = (pc, pmt, cxs, rr)
                for th in drip.get(i, ()):
                    th()
                if nb is not None:
                    bca, bcb = norm_bcast(nb[2], nb[3], use_pe=False)
                if i + 2 < len(steps):
                    emit_sc(i + 2)
                if nb is not None:
                    norm_mul(nb[0], nb[1], nb[2], bca, bcb)
                cxt = cx_t[(c, mt)]
                for hp in range(2):
                    nc.tensor.matmul(
                        cxt[0:65, hp * 512 : (hp + 1) * 512],
                        v_sb[:, kj, 2 * mt + hp, :],
                        at[:, hp * 512 : (hp + 1) * 512],
                        start=(kj == 0),
                        stop=(kj == KJT - 1),
                    )

            # tail: last phase's normalize, split into qi halves (copies on
            # ACT which is idle now) so out-proj pieces start ~1.5us earlier
            cxt = cx_t.pop((NCH - 1, 1))
            cxs = npl.tile([128, 1024], f32, tag="cxs")
            rt = npl.tile([128, 1024], f32, tag="rt")
            rr = npl.tile([128, 1024], f32, tag="rr")
            bc = npl.tile([128, 1024], f32, tag="bcs")
            col0 = (NCH - 1) * 512

            def gview(t):
                return t.rearrange("p (g q) -> p g q", g=2)

            def tail_norm_half(h):
                hsl = slice(256 * h, 256 * h + 256)
                nc.scalar.copy(gview(cxs)[0:65, :, hsl], gview(cxt)[0:65, :, hsl])
                nc.vector.tensor_copy(
                    gview(rt)[0:1, :, hsl], gview(cxs)[64:65, :, hsl]
                )
                nc.vector.reciprocal_approx_fast(
                    out=gview(rr)[0:1, :, hsl], in_=gview(rt)[0:1, :, hsl]
                )
                nc.gpsimd.partition_broadcast(
                    gview(bc)[0:64, :, hsl], gview(rr)[0:1, :, hsl]
                )
                ccol = slice(col0 + 256 * h, col0 + 256 * h + 256)
                nc.vector.tensor_mul(
                    ctxT_sb[0:64, 1, ccol], cxs[0:64, hsl], bc[0:64, hsl]
                )
                for q in range(2):
                    qs = slice(32 * q, 32 * q + 32)
                    h1 = slice(512 + 256 * h, 512 + 256 * h + 256)
                    nc.vector.tensor_mul(
                        ctxT_sb[64 + 32 * q : 96 + 32 * q, 1, ccol],
                        cxs[qs, h1],
                        bc[qs, h1],
                    )

            tail_norm_half(0)
            tail_norm_half(1)
            for st in range(4):
                for nt in range(2):
                    out_piece(3, st, nt, tail=True)

    nc.compile()
    return nc


def _get_nc():
    global _CACHED_NC
    if _CACHED_NC is None:
        _CACHED_NC = _build()
    return _CACHED_NC


def _pack_w_half(W, g, mt):
    """[128, 8, 128] f16: [p, kt, col] = W[kt*128+p, g*256+mt*128+col]."""
    sl = W[:, g * DL + mt * 128 : g * DL + (mt + 1) * 128]
    return np.ascontiguousarray(
        sl.reshape(KT, 128, 128).transpose(1, 0, 2).astype(np.float16)
    )


def _in_maps(x, Wq, bq, Wk, bk, Wv, bv, Wo, bo):
    xpacks = []
    for b in range(B):
        xpacks.append(
            np.ascontiguousarray(
                x[b]
                .reshape(NCH, 512, KT, 128)
                .transpose(3, 0, 2, 1)
                .astype(np.float16)
            )
        )
    maps = []
    for c in range(N_CORES):
        b, g = c // 4, c % 4
        cs = slice(g * DL, (g + 1) * DL)
        maps.append(
            {
                "xp": xpacks[b],
                "wq0": _pack_w_half(Wq, g, 0),
                "wq1": _pack_w_half(Wq, g, 1),
                "wk0": _pack_w_half(Wk, g, 0),
                "wk1": _pack_w_half(Wk, g, 1),
                "wv0": _pack_w_half(Wv, g, 0),
                "wv1": _pack_w_half(Wv, g, 1),
                "wo": np.ascontiguousarray(
                    Wo[cs, :]
                    .reshape(2, 128, D)
                    .transpose(1, 0, 2)
                    .astype(np.float16)
                ),
                "bq": np.ascontiguousarray(bq[cs].reshape(2, 128).T.astype(np.float32)),
                "bk": np.ascontiguousarray(bk[cs].reshape(2, 128).T.astype(np.float32)),
            }
        )
    return maps


def _assemble(results, bv, Wo, bo):
    corr = (bv.astype(np.float64) @ Wo.astype(np.float64)) + bo.astype(np.float64)
    outs = []
    for b in range(B):
        acc = np.zeros((S, D), dtype=np.float64)
        for g in range(4):
            acc += results[b * 4 + g]["out"].astype(np.float64)
        outs.append((acc + corr).astype(np.float32))
    return np.stack(outs)


def kernel(x, Wq, bq, Wk, bk, Wv, bv, Wo, bo):
    from concourse.bass_utils import run_bass_kernel_spmd

    x = np.asarray(x, dtype=np.float32)
    Wq = np.asarray(Wq, dtype=np.float32)
    Wk = np.asarray(Wk, dtype=np.float32)
    Wv = np.asarray(Wv, dtype=np.float32)
    Wo = np.asarray(Wo, dtype=np.float32)
    bq = np.asarray(bq, dtype=np.float32)
    bk = np.asarray(bk, dtype=np.float32)
    bv = np.asarray(bv, dtype=np.float32)
    bo = np.asarray(bo, dtype=np.float32)

    nc = _get_nc()
    res = run_bass_kernel_spmd(
        nc, _in_maps(x, Wq, bq, Wk, bk, Wv, bv, Wo, bo), core_ids=list(range(N_CORES))
    )
    return _assemble(res.results, bv, Wo, bo)
